# revision 2
# baseline (speedup 1.0000x reference)
"""Canny edge detector (nn_CannyNet) on 8 Trainium2 NeuronCores.

Self-contained: hardcodes shapes [4,3,1024,1024] and the filter constants.

Decomposition: 27 units = (3 channels) x (9 row-blocks: 8x120 + 1x64 rows).
Uniform SPMD program: every core processes 4 units (5 cores repeat their
first unit; host ignores the duplicate). Per unit, the 4 batch planes of one
channel are processed together because the reference's flat NMS gather
couples batches.

Math (derived from the reference, sqrt/atan2/gather eliminated):
  x = img + 1 (host pad = -1 so pads become 0; 127.5 scale folded into T')
  H = hgauss(x)/g1 (free-dim shifts, DVE/GPSIMD)
  cx = (S121 @ G) H, cyn = (S101 @ G) H  (PE banded matmuls, boundary-exact)
  gx = cx(j-1)-cx(j+1), gy = 2*cyn + cyn(j-1)+cyn(j+1)
  m2 = gx^2+gy^2  (~ grad_mag^2 scaled by (127.5*g1^2)^-2)
  out(b) = (m2_b >= T') & [m2_B1 > m2_B1@(sgn*o_b)] & [m2_B2 > ...]
    where o_b = offset of direction b (E/SE/S/SW), sgn=-1 on diagonal axes,
    (B1,B2) = (0,2) for axes {0,1} else (1,3), axis from gx/gy comparisons.
  Row-shifted m2 views via SBUF->SBUF DMA copies (m2p/m2m).
"""
import math
import os
import numpy as np

import concourse.bass as bass
import concourse.mybir as mybir
from concourse.bass_utils import run_bass_kernel_spmd

ALU = mybir.AluOpType
AF = mybir.ActivationFunctionType
DT = mybir.dt.float32
F16 = mybir.dt.float16
U16 = mybir.dt.uint16

B, C, H_IMG, W = 4, 3, 1024, 1024
NU = 1 if os.environ.get('KDBG') else 4  # units per core (uniform)
M = 122           # m2/out row span per unit (out 120 + 2)
XR = 128          # x-tile rows
FW = 1028         # x-tile width (cols -2..1025)
MW = 1026         # m2 width (cols -1..1024)

_g = np.exp(-0.5 * np.arange(-2, 3, dtype=np.float64) ** 2)
G1 = _g[1]
R0 = float(np.float32(_g[0] / _g[1]))   # g0/g1
R2 = float(np.float32(1.0 / _g[1]))     # 1/g1
# kernel chain: H scaled 1/g1 (hgauss), bands unnormalized -> m2 = m2_ref/(127.5*g1)^2
THR = float(np.float32((400.0 / (127.5 * G1)) ** 2))
_t1 = math.tan(22.5 * 3.14159 / 180.0)
_t3 = math.tan(67.5 * 3.14159 / 180.0)
T1SQ = float(np.float32(_t1 * _t1))
T3SQ = float(np.float32(_t3 * _t3))

# units and core assignment
UNITS = [(c, k) for k in range(9) for c in range(3)]  # 27
CORE_UNITS = []
for i in range(8):
    us = [UNITS[i], UNITS[i + 8], UNITS[i + 16]]
    us.append(UNITS[24 + i] if i < 3 else UNITS[i])  # dummy repeat for cores 3..7
    CORE_UNITS.append(us)


def _unit_rows(k):
    """(xbase, out0): x-tile img rows xbase..xbase+127; out rows out0..out0+119
    (k=8: only first 64 valid)."""
    if k < 8:
        return 120 * k - 4, 120 * k
    return 900, 960


def _make_bands():
    """CX = S121 @ G, CY = S101 @ G over image rows with zero-pad truncation."""
    n = H_IMG
    G = np.zeros((n, n), np.float64)
    for kk in range(-2, 3):
        v = _g[kk + 2]
        for o in range(max(0, -kk), min(n, n - kk)):
            G[o, o + kk] = v
    S121 = np.zeros((n, n), np.float64)
    S101 = np.zeros((n, n), np.float64)
    for o in range(n):
        for kk, w1, w2 in ((-1, 1.0, 1.0), (0, 2.0, 0.0), (1, 1.0, -1.0)):
            i = o + kk
            if 0 <= i < n:
                S121[o, i] = w1
                if kk != 0:
                    S101[o, i] = w2
    CX = (S121 @ G).astype(np.float32)
    CY = (S101 @ G).astype(np.float32)
    return CX, CY


def _band_lhsT(Cm, k):
    """lhsT [XR, M]: lhsT[kr, m] = Cm[out0-1+m, xbase+kr] (0 out of range)."""
    xbase, out0 = _unit_rows(k)
    out = np.zeros((XR, M), np.float32)
    for m in range(M):
        orow = out0 - 1 + m
        if not (0 <= orow < H_IMG):
            continue
        for d in range(-3, 4):
            irow = orow + d
            kr = irow - xbase
            if 0 <= irow < H_IMG and 0 <= kr < XR:
                out[kr, m] = Cm[orow, irow]
    return out


def build_nc():
    nc = bass.Bass()
    xin = nc.declare_dram_parameter("xin", [NU, B, XR, FW], DT, isOutput=False)
    bands = nc.declare_dram_parameter("bands", [128, NU * 2 * M], DT, isOutput=False)
    outd = nc.declare_dram_parameter("out", [NU, B, 120, W], DT, isOutput=True)
    dbg = None
    if os.environ.get('KDBG'):
        dbg = {
            "dbg_m2": nc.declare_dram_parameter("dbg_m2", [B, M, MW], DT, isOutput=True),
            "dbg_m2p": nc.declare_dram_parameter("dbg_m2p", [B, M, MW], DT, isOutput=True),
            "dbg_m2m": nc.declare_dram_parameter("dbg_m2m", [B, M, MW], DT, isOutput=True),
            "dbg_h": nc.declare_dram_parameter("dbg_h", [XR, W], DT, isOutput=True),
            "dbg_gy": nc.declare_dram_parameter("dbg_gy", [M, W], DT, isOutput=True),
        }

    from contextlib import ExitStack
    es = ExitStack()
    ent = es.enter_context

    x = [ent(nc.sbuf_tensor(f"x{b}", [XR, FW], DT)) for b in range(B)]
    s1 = [ent(nc.sbuf_tensor(f"s1_{b}", [XR, W], DT)) for b in range(B)]
    s2 = [ent(nc.sbuf_tensor(f"s2_{b}", [XR, W], DT)) for b in range(B)]
    u_t = ent(nc.sbuf_tensor("u_t", [XR, W], DT))
    Ht = [ent(nc.sbuf_tensor(f"Ht{b}", [XR, W], DT)) for b in range(B)]
    tcx = [ent(nc.sbuf_tensor(f"tcx{b}", [M, W], DT)) for b in range(B)]  # cx, later sqx
    tcy = [ent(nc.sbuf_tensor(f"tcy{b}", [M, W], DT)) for b in range(B)]  # cyn, later sqy
    gx = [ent(nc.sbuf_tensor(f"gx{b}", [M, W], DT)) for b in range(B)]
    # gy aliased into s1 (s1 dead after u_t; safe transitively via s_sq->m2 order)
    gy = [s1[b][0:M, :] for b in range(B)]
    f1_t = ent(nc.sbuf_tensor("f1_t", [M, W], DT))
    sp_t = ent(nc.sbuf_tensor("sp_t", [M, W], DT))
    m2 = [ent(nc.sbuf_tensor(f"m2_{b}", [M, MW], DT)) for b in range(B)]
    m2p = [ent(nc.sbuf_tensor(f"m2p{b}", [M, MW], DT)) for b in range(B)]
    m2m = [ent(nc.sbuf_tensor(f"m2m{b}", [M, MW], DT)) for b in range(B)]
    pre_t = ent(nc.sbuf_tensor("pre_t", [M, W], DT))
    mh_t = ent(nc.sbuf_tensor("mh_t", [M, W], U16))
    mv_t = ent(nc.sbuf_tensor("mv_t", [M, W], U16))
    sel1 = ent(nc.sbuf_tensor("sel1", [M, W], F16))
    sel2 = ent(nc.sbuf_tensor("sel2", [M, W], F16))
    cA = ent(nc.sbuf_tensor("cA", [M, W], F16))
    cB = ent(nc.sbuf_tensor("cB", [M, W], F16))
    cC = ent(nc.sbuf_tensor("cC", [M, W], F16))
    s12 = ent(nc.sbuf_tensor("s12", [M, W], F16))
    # d1 masks live in the (dead-by-then) first half of gx[b]'s bytes
    d1v = [gx[b].bitcast(U16)[:, 0:W] for b in range(B)]
    bnd = ent(nc.sbuf_tensor("bnd", [128, NU * 2 * M], DT))
    out_t = [ent(nc.sbuf_tensor(f"out_t{j}", [M, W], DT)) for j in range(2)]
    pcx = [ent(nc.psum_tensor(f"pcx{j}", [M, W], DT)) for j in range(2)]
    pcy = [ent(nc.psum_tensor(f"pcy{j}", [M, W], DT)) for j in range(2)]

    d_x = ent(nc.semaphore("d_x"))
    d_b = ent(nc.semaphore("d_b"))
    d_sh = ent(nc.semaphore("d_sh"))
    d_out = ent(nc.semaphore("d_out"))
    g_pre = ent(nc.semaphore("g_pre"))
    g_f1 = ent(nc.semaphore("g_f1"))
    g_sp = ent(nc.semaphore("g_sp"))
    g_m2 = ent(nc.semaphore("g_m2"))
    v_h = ent(nc.semaphore("v_h"))
    v_sob = ent(nc.semaphore("v_sob"))
    v_msk = ent(nc.semaphore("v_msk"))
    v_d1 = ent(nc.semaphore("v_d1"))
    v_fin = ent(nc.semaphore("v_fin"))
    s_evac = ent(nc.semaphore("s_evac"))
    s_sq = ent(nc.semaphore("s_sq"))
    pe = ent(nc.semaphore("pe"))
    block = ent(nc.Block())

    def IX(u, b):
        return 4 * u + b + 1  # 1-based cumulative count at completion of (u,b)

    @block.sync
    def _(sync):
        sync.dma_start(out=bnd[:], in_=bands[:]).then_inc(d_b, 16)
        for u in range(NU):
            for b in range(B):
                # x[b] WAR: V.H(u-1,b) read x
                if u > 0:
                    sync.wait_ge(v_h, IX(u - 1, b))
                sync.dma_start(out=x[b][:], in_=xin[u, b]).then_inc(d_x, 16)
            for b in range(B):
                # m2 shifts: after G.m2(u,b); WAR vs V cmps of u-1
                sync.wait_ge(g_m2, IX(u, b))
                if u > 0:
                    sync.wait_ge(v_fin, IX(u - 1, 3))
                sync.dma_start(out=m2p[b][0:M - 1, :],
                               in_=m2[b][1:M, :]).then_inc(d_sh, 16)
                sync.dma_start(out=m2m[b][1:M, :],
                               in_=m2[b][0:M - 1, :]).then_inc(d_sh, 16)
            for b in range(B):
                sync.wait_ge(v_fin, IX(u, b))
                sync.dma_start(out=outd[u, b],
                               in_=out_t[(4 * u + b) % 2][1:121, :]).then_inc(d_out, 16)
        if dbg is not None:
            sync.wait_ge(v_fin, NU * B)
            for b in range(B):
                sync.dma_start(out=dbg["dbg_m2"][b], in_=m2[b][:]).then_inc(d_out, 16)
                sync.dma_start(out=dbg["dbg_m2p"][b], in_=m2p[b][:]).then_inc(d_out, 16)
                sync.dma_start(out=dbg["dbg_m2m"][b], in_=m2m[b][:]).then_inc(d_out, 16)
            sync.dma_start(out=dbg["dbg_h"][:], in_=Ht[0][:]).then_inc(d_out, 16)
            sync.dma_start(out=dbg["dbg_gy"][:], in_=gy[0]).then_inc(d_out, 16)
            sync.wait_ge(d_out, 16 * (NU * B + 14))
        else:
            sync.wait_ge(d_out, 16 * NU * B)

    @block.gpsimd
    def _(gpsimd):
        # prologue: zero m2 pad cols (cols 0 and 1025) once
        for b in range(B):
            gpsimd.memset(m2[b][:, 0:1], 0.0)
            gpsimd.memset(m2[b][:, 1025:1026], 0.0)
        for u in range(NU):
            for b in range(B):
                gpsimd.wait_ge(d_x, 16 * IX(u, b))
                # affine in place: x += 1
                gpsimd.tensor_scalar(out=x[b][:], in0=x[b][:], scalar1=1.0,
                                     scalar2=None, op0=ALU.add)
                # s1 = x<<1 + x>>1 ; s2 = x<<2 + x>>2 (image cols 0..1023)
                gpsimd.tensor_tensor(out=s1[b][:], in0=x[b][:, 1:1025],
                                     in1=x[b][:, 3:1027], op=ALU.add)
                gpsimd.tensor_tensor(out=s2[b][:], in0=x[b][:, 0:1024],
                                     in1=x[b][:, 4:1028],
                                     op=ALU.add).then_inc(g_pre, 1)
            for b in range(B):
                # f1 = cyn(j-1)+cyn(j+1); borders f1[0]=cyn[1], f1[1023]=cyn[1022]
                gpsimd.wait_ge(s_evac, IX(u, b))
                if u > 0 or b > 0:
                    gpsimd.wait_ge(v_sob, IX(u, b) - 1)  # f1_t WAR
                gpsimd.tensor_tensor(out=f1_t[:, 1:1023], in0=tcy[b][:, 0:1022],
                                     in1=tcy[b][:, 2:1024], op=ALU.add)
                gpsimd.tensor_scalar(out=f1_t[:, 0:1], in0=tcy[b][:, 1:2],
                                     scalar1=0.0, scalar2=None, op0=ALU.add)
                gpsimd.tensor_scalar(out=f1_t[:, 1023:1024], in0=tcy[b][:, 1022:1023],
                                     scalar1=0.0, scalar2=None,
                                     op0=ALU.add).then_inc(g_f1, 1)
                # sp = gx*gy (needs V gx/gy of this plane; sp_t WAR vs V.d1 prior)
                gpsimd.wait_ge(v_sob, IX(u, b))
                if u > 0 or b > 0:
                    gpsimd.wait_ge(v_d1, IX(u, b) - 1)
                gpsimd.tensor_tensor(out=sp_t[:], in0=gx[b][:], in1=gy[b],
                                     op=ALU.mult).then_inc(g_sp, 1)
                # m2 = sqx + sqy (into cols 1..1024)
                gpsimd.wait_ge(s_sq, IX(u, b))
                if u > 0:
                    gpsimd.wait_ge(v_fin, IX(u - 1, b))  # m2[b] WAR
                gpsimd.tensor_tensor(out=m2[b][:, 1:1025], in0=tcx[b][:],
                                     in1=tcy[b][:], op=ALU.add).then_inc(g_m2, 1)

    @block.vector
    def _(vector):
        for u in range(NU):
            for b in range(B):
                vector.wait_ge(g_pre, IX(u, b))
                if u > 0:
                    vector.wait_ge(pe, IX(u - 1, b))  # H[b] WAR
                # u_t = s2*R0 + s1 ; H = x*R2 + u_t
                vector.scalar_tensor_tensor(out=u_t[:], in0=s2[b][:], scalar=R0,
                                            in1=s1[b][:], op0=ALU.mult, op1=ALU.add)
                vector.scalar_tensor_tensor(out=Ht[b][:], in0=x[b][:, 2:1026],
                                            scalar=R2, in1=u_t[:], op0=ALU.mult,
                                            op1=ALU.add).then_inc(v_h, 1)
            for b in range(B):
                # hsobel: gx = cx(j-1) - cx(j+1); borders
                vector.wait_ge(s_evac, IX(u, b))
                if u > 0:
                    vector.wait_ge(s_sq, IX(u - 1, b))  # gx/gy WAR (sq read them)
                    vector.wait_ge(g_sp, IX(u - 1, b))
                vector.tensor_tensor(out=gx[b][:, 1:1023], in0=tcx[b][:, 0:1022],
                                     in1=tcx[b][:, 2:1024], op=ALU.subtract)
                vector.tensor_scalar(out=gx[b][:, 0:1], in0=tcx[b][:, 1:2],
                                     scalar1=-1.0, scalar2=None, op0=ALU.mult)
                vector.tensor_scalar(out=gx[b][:, 1023:1024], in0=tcx[b][:, 1022:1023],
                                     scalar1=1.0, scalar2=None, op0=ALU.mult)
                # gy = 2*cyn + f1
                vector.wait_ge(g_f1, IX(u, b))
                vector.scalar_tensor_tensor(out=gy[b], in0=tcy[b][:], scalar=2.0,
                                            in1=f1_t[:], op0=ALU.mult,
                                            op1=ALU.add).then_inc(v_sob, 1)
                vector.wait_ge(g_sp, IX(u, b))
                vector.wait_ge(s_sq, IX(u, b))
                vector.tensor_scalar(out=d1v[b][:], in0=sp_t[:], scalar1=0.0,
                                     scalar2=None,
                                     op0=ALU.is_gt).then_inc(v_d1, 1)
            # NMS
            vector.wait_ge(d_sh, 32 * 4 * (u + 1))  # all 4 planes shifted
            for b in range(B):
                # masks from squares (in tcx/tcy) and sp
                vector.wait_ge(s_sq, IX(u, b))
                vector.tensor_scalar(out=pre_t[:], in0=tcx[b][:], scalar1=T1SQ,
                                     scalar2=None, op0=ALU.mult)
                vector.tensor_tensor(out=mh_t[:], in0=pre_t[:], in1=tcy[b][:],
                                     op=ALU.is_ge)
                vector.tensor_scalar(out=pre_t[:], in0=tcx[b][:], scalar1=T3SQ,
                                     scalar2=None, op0=ALU.mult)
                vector.tensor_tensor(out=mv_t[:], in0=tcy[b][:], in1=pre_t[:],
                                     op=ALU.is_ge).then_inc(v_msk, 1)
                # cmp in1 views per b: +o and -o
                if b == 0:
                    pv = [m2[q][:, 2:1026] for q in range(B)]
                    mv_ = [m2[q][:, 0:1024] for q in range(B)]
                elif b == 1:
                    pv = [m2p[q][:, 2:1026] for q in range(B)]
                    mv_ = [m2m[q][:, 0:1024] for q in range(B)]
                elif b == 2:
                    pv = [m2p[q][:, 1:1025] for q in range(B)]
                    mv_ = [m2m[q][:, 1:1025] for q in range(B)]
                else:
                    pv = [m2p[q][:, 0:1024] for q in range(B)]
                    mv_ = [m2m[q][:, 2:1026] for q in range(B)]
                ctr = [m2[q][:, 1:1025] for q in range(B)]
                # sel1 cascade over beta in {0,1}; sel2 over {2,3}
                for sel, b1, b2 in ((sel1, 0, 1), (sel2, 2, 3)):
                    vector.tensor_tensor(out=sel[:], in0=ctr[b2], in1=mv_[b2],
                                         op=ALU.is_gt)
                    vector.tensor_tensor(out=cA[:], in0=ctr[b1], in1=mv_[b1],
                                         op=ALU.is_gt)
                    vector.tensor_tensor(out=cB[:], in0=ctr[b2], in1=pv[b2],
                                         op=ALU.is_gt)
                    vector.tensor_tensor(out=cC[:], in0=ctr[b1], in1=pv[b1],
                                         op=ALU.is_gt)
                    vector.copy_predicated(out=sel[:], mask=d1v[b][:], data=cA[:])
                    vector.copy_predicated(out=sel[:], mask=mv_t[:], data=cB[:])
                    vector.copy_predicated(out=sel[:], mask=mh_t[:], data=cC[:])
                vector.tensor_tensor(out=s12[:], in0=sel1[:], in1=sel2[:],
                                     op=ALU.mult)
                ot = out_t[(4 * u + b) % 2]
                if 4 * u + b >= 2:
                    vector.wait_ge(d_out, 16 * (IX(u, b) - 2))
                vector.scalar_tensor_tensor(out=ot[:], in0=m2[b][:, 1:1025],
                                            scalar=THR, in1=s12[:], op0=ALU.is_ge,
                                            op1=ALU.mult).then_inc(v_fin, 1)

    @block.scalar
    def _(scalar):
        for u in range(NU):
            done_sq = 0
            for b in range(B):
                scalar.wait_ge(pe, IX(u, b))
                if u > 0:
                    scalar.wait_ge(v_msk, IX(u - 1, b))  # tcx/tcy WAR (masks read sq)
                    scalar.wait_ge(g_m2, IX(u - 1, b))
                nc.scalar.copy(out=tcx[b][:], in_=pcx[b % 2][:])
                nc.scalar.copy(out=tcy[b][:], in_=pcy[b % 2][:]).then_inc(s_evac, 1)
                while done_sq < b:  # interleave squares once inputs ready
                    q = done_sq
                    scalar.wait_ge(v_sob, IX(u, q))
                    scalar.wait_ge(g_sp, IX(u, q))
                    nc.scalar.activation(out=tcx[q][:], in_=gx[q][:], func=AF.Square)
                    nc.scalar.activation(out=tcy[q][:], in_=gy[q],
                                         func=AF.Square).then_inc(s_sq, 1)
                    done_sq += 1
            for q in range(done_sq, B):
                scalar.wait_ge(v_sob, IX(u, q))
                scalar.wait_ge(g_sp, IX(u, q))
                nc.scalar.activation(out=tcx[q][:], in_=gx[q][:], func=AF.Square)
                nc.scalar.activation(out=tcy[q][:], in_=gy[q],
                                     func=AF.Square).then_inc(s_sq, 1)

    @block.tensor
    def _(tensor):
        tensor.wait_ge(d_b, 16)
        for u in range(NU):
            bx = bnd[:, (u * 2 + 0) * M:(u * 2 + 1) * M]
            by = bnd[:, (u * 2 + 1) * M:(u * 2 + 2) * M]
            for b in range(B):
                tensor.wait_ge(v_h, IX(u, b))
                if 4 * u + b >= 2:
                    tensor.wait_ge(s_evac, IX(u, b) - 2)  # PSUM pair reuse
                p, q = pcx[b % 2], pcy[b % 2]
                nc.tensor.matmul(p[:, 0:512], bx, Ht[b][:, 0:512],
                                 start=True, stop=True)
                nc.tensor.matmul(p[:, 512:1024], bx, Ht[b][:, 512:1024],
                                 start=True, stop=True)
                nc.tensor.matmul(q[:, 0:512], by, Ht[b][:, 0:512],
                                 start=True, stop=True)
                nc.tensor.matmul(q[:, 512:1024], by, Ht[b][:, 512:1024],
                                 start=True, stop=True).then_inc(pe, 1)

    es.close()
    return nc


_NC_CACHE = {}


def kernel(img, gauss_h=None, gauss_v=None, sobel_h=None, sobel_v=None,
           dir_w=None, **_):
    img = np.asarray(img, dtype=np.float32)
    assert img.shape == (B, C, H_IMG, W)

    # host pad with -1 (affine +1 makes pads exactly 0)
    pad = np.full((B, C, H_IMG + 8, W + 4), -1.0, np.float32)
    pad[:, :, 4:4 + H_IMG, 2:2 + W] = img

    CX, CY = _make_bands()
    band_cache = {}
    for c, k in UNITS:
        if k not in band_cache:
            band_cache[k] = (_band_lhsT(CX, k), _band_lhsT(CY, k))

    in_maps = []
    for i in range(8):
        xin = np.empty((NU, B, XR, FW), np.float32)
        bands = np.zeros((128, NU * 2 * M), np.float32)
        for u, (c, k) in enumerate(CORE_UNITS[i]):
            xbase, _o = _unit_rows(k)
            r = xbase + 4  # padded row index
            for b in range(B):
                xin[u, b] = pad[b, c, r:r + XR, :]
            bx, by = band_cache[k]
            bands[:, (u * 2) * M:(u * 2 + 1) * M] = bx
            bands[:, (u * 2 + 1) * M:(u * 2 + 2) * M] = by
        in_maps.append({"xin": xin, "bands": bands})

    key = "nc"
    if key not in _NC_CACHE:
        _NC_CACHE[key] = build_nc()
    nc = _NC_CACHE[key]
    r = run_bass_kernel_spmd(nc, in_maps, list(range(8)))
    globals()["LAST_RESULT"] = r
    res = r.results

    out = np.zeros((B, C, H_IMG, W), np.float32)
    for i in range(8):
        for u, (c, k) in enumerate(CORE_UNITS[i]):
            if i >= 3 and u == 3:
                continue  # dummy repeat
            _xb, out0 = _unit_rows(k)
            rows = 120 if k < 8 else 64
            out[:, c, out0:out0 + rows, :] = res[i]["out"][u, :, :rows, :]
    mn, mx = out.min(), out.max()
    return ((out - mn) / (mx - mn)).astype(np.float32)



# revision 11
# speedup vs baseline: 1.7492x; 1.7492x over previous
"""Canny edge detector (nn_CannyNet) on 8 Trainium2 NeuronCores.

Self-contained: hardcodes shapes [4,3,1024,1024] and the filter constants.

Decomposition: 27 units = (3 channels) x (9 row-blocks: 8x120 + 1x64 rows).
Uniform SPMD program: every core processes 4 units (5 cores repeat their
first unit; host ignores the duplicate). Per unit, the 4 batch planes of one
channel are processed together because the reference's flat NMS gather
couples batches.

Engine split (per plane, [<=128, 1024] tiles):
  Pool : s1/s2 col shift-adds, u=v+s1, H=u+w, m2=sqx+sqy (plain TT adds only)
  DVE  : v=s2*R0, w=x*R2 (fast TS), masks + fp16 NMS compare cascade
  PE   : gx/gy directly via accumulating banded matmuls with col-shifted
         rhs views of the zero-padded H tile (vertical conv x horizontal taps)
  Act  : squares (with tan-scale folding) + signs straight from PSUM, fp16 out
All NMS compares run in fp16 (2x DVE rate), planes packed in pairs (1,3)/(0,2)
so one compare instruction covers both planes of an AND-candidate.
Row-shifted m2 views (m2p/m2m) via SBUF->SBUF DMA (engines cannot shift
partitions). Final out is fp16 0/1, converted on host.
"""
import math
import os
import numpy as np

import concourse.bass as bass
import concourse.mybir as mybir
from concourse.bass_utils import run_bass_kernel_spmd

ALU = mybir.AluOpType
AF = mybir.ActivationFunctionType
DT = mybir.dt.float32
F16 = mybir.dt.float16
U16 = mybir.dt.uint16

B, C, H_IMG, W = 4, 3, 1024, 1024
NU = 1 if os.environ.get('KDBG') else 4  # units per core (uniform)
M = 122           # m2/out row span per unit (out 120 + 2)
XR = 128          # x-tile rows
FW = 1028         # x-tile width (cols -2..1025)
HW2 = 1026        # H tile width (cols -1..1024, zero side cols)
MW = 1026         # m2 width per plane (cols -1..1024)

_g = np.exp(-0.5 * np.arange(-2, 3, dtype=np.float64) ** 2)
G1 = _g[1]
R0 = float(np.float32(_g[0] / _g[1]))   # g0/g1
R2 = float(np.float32(1.0 / _g[1]))     # 1/g1
THR = float(np.float32((400.0 / (127.5 * G1)) ** 2))
_t1 = math.tan(22.5 * 3.14159 / 180.0)
_t3 = math.tan(67.5 * 3.14159 / 180.0)

# units and core assignment
UNITS = [(c, k) for k in range(9) for c in range(3)]  # 27
CORE_UNITS = []
for i in range(8):
    us = [UNITS[i], UNITS[i + 8], UNITS[i + 16]]
    us.append(UNITS[24 + i] if i < 3 else UNITS[i])  # dummy repeat for cores 3..7
    CORE_UNITS.append(us)


def _unit_rows(k):
    """(xbase, out0): x-tile img rows xbase..xbase+127; out rows out0..out0+119
    (k=8: only first 64 valid)."""
    if k < 8:
        return 120 * k - 4, 120 * k
    return 900, 960


def _make_bands():
    """CX = S121 @ G, CY = S101 @ G over image rows with zero-pad truncation."""
    n = H_IMG
    G = np.zeros((n, n), np.float64)
    for kk in range(-2, 3):
        v = _g[kk + 2]
        for o in range(max(0, -kk), min(n, n - kk)):
            G[o, o + kk] = v
    S121 = np.zeros((n, n), np.float64)
    S101 = np.zeros((n, n), np.float64)
    for o in range(n):
        for kk, w1, w2 in ((-1, 1.0, 1.0), (0, 2.0, 0.0), (1, 1.0, -1.0)):
            i = o + kk
            if 0 <= i < n:
                S121[o, i] = w1
                if kk != 0:
                    S101[o, i] = w2
    CX = (S121 @ G).astype(np.float32)
    CY = (S101 @ G).astype(np.float32)
    return CX, CY


def _band_lhsT(Cm, k):
    """lhsT [XR, M]: lhsT[kr, m] = Cm[out0-1+m, xbase+kr] (0 out of range)."""
    xbase, out0 = _unit_rows(k)
    out = np.zeros((XR, M), np.float32)
    for m in range(M):
        orow = out0 - 1 + m
        if not (0 <= orow < H_IMG):
            continue
        for d in range(-3, 4):
            irow = orow + d
            kr = irow - xbase
            if 0 <= irow < H_IMG and 0 <= kr < XR:
                out[kr, m] = Cm[orow, irow]
    return out


def build_nc():
    nc = bass.Bass()
    xin = nc.declare_dram_parameter("xin", [NU, B, XR, FW], DT, isOutput=False)
    bands = nc.declare_dram_parameter("bands", [128, NU * 4 * M], DT, isOutput=False)
    outd = nc.declare_dram_parameter("out", [NU, B, 120, W], F16, isOutput=True)
    dbg = None
    if os.environ.get('KDBG'):
        dbg = {
            "dbg_t02": nc.declare_dram_parameter("dbg_t02", [M, 2, MW], DT, isOutput=True),
            "dbg_t13": nc.declare_dram_parameter("dbg_t13", [M, 2, MW], DT, isOutput=True),
            "dbg_mhv": nc.declare_dram_parameter("dbg_mhv", [B, M, 2 * W], U16, isOutput=True),
            "dbg_d1m": nc.declare_dram_parameter("dbg_d1m", [B, M, W], U16, isOutput=True),
            "dbg_h": nc.declare_dram_parameter("dbg_h", [2, XR, HW2], DT, isOutput=True),
            "dbg_sq": nc.declare_dram_parameter("dbg_sq", [2, M, 3 * W], F16, isOutput=True),
        }

    from contextlib import ExitStack
    es = ExitStack()
    ent = es.enter_context

    x = [ent(nc.sbuf_tensor(f"x{b}", [XR, FW], DT)) for b in range(B)]
    s1 = [ent(nc.sbuf_tensor(f"s1_{j}", [XR, W], DT)) for j in range(2)]
    s2 = [ent(nc.sbuf_tensor(f"s2_{j}", [XR, W], DT)) for j in range(2)]
    vv = [ent(nc.sbuf_tensor(f"vv{j}", [XR, W], DT)) for j in range(2)]
    ww = [ent(nc.sbuf_tensor(f"ww{j}", [XR, W], DT)) for j in range(2)]
    Ht = [ent(nc.sbuf_tensor(f"Ht{j}", [XR, HW2], DT)) for j in range(2)]
    ut = s2  # ut aliases s2: s2[j] is dead once DVE v(b) = s2*R0 has read it
    # fp16 working set
    sqa = [ent(nc.sbuf_tensor(f"sqa{j}", [M, 3 * W], F16)) for j in range(2)]
    tsqx = [ent(nc.sbuf_tensor(f"tsqx{j}", [M, W], DT)) for j in range(2)]
    tsqy = [ent(nc.sbuf_tensor(f"tsqy{j}", [M, W], DT)) for j in range(2)]
    sgx = [ent(nc.sbuf_tensor(f"sgx{j}", [M, W], F16)) for j in range(2)]
    sgy = [ent(nc.sbuf_tensor(f"sgy{j}", [M, W], F16)) for j in range(2)]
    mhv = [ent(nc.sbuf_tensor(f"mhv{b}", [M, 2 * W], U16)) for b in range(B)]
    d1m = [ent(nc.sbuf_tensor(f"d1m{b}", [M, W], U16)) for b in range(B)]
    # packed m2 tiles: [M, 2, MW] — slice 0/1 = planes (0,2) in t02, (1,3) in t13
    t02 = ent(nc.sbuf_tensor("t02", [M, 2, MW], DT))
    t13 = ent(nc.sbuf_tensor("t13", [M, 2, MW], DT))
    t02p = ent(nc.sbuf_tensor("t02p", [M, 2, MW], DT))
    t02m = ent(nc.sbuf_tensor("t02m", [M, 2, MW], DT))
    t13p = ent(nc.sbuf_tensor("t13p", [M, 2, MW], DT))
    t13m = ent(nc.sbuf_tensor("t13m", [M, 2, MW], DT))
    cn13 = ent(nc.sbuf_tensor("cn13", [M, 2, W], F16))
    cn02 = ent(nc.sbuf_tensor("cn02", [M, 2, W], F16))
    cp13 = ent(nc.sbuf_tensor("cp13", [M, 2, W], F16))
    cp02 = ent(nc.sbuf_tensor("cp02", [M, 2, W], F16))
    s12 = ent(nc.sbuf_tensor("s12", [M, W], F16))
    aD = ent(nc.sbuf_tensor("aD", [M, W], F16))
    aV = ent(nc.sbuf_tensor("aV", [M, W], F16))
    aH = ent(nc.sbuf_tensor("aH", [M, W], F16))
    thr_t = ent(nc.sbuf_tensor("thr_t", [M, W], F16))
    out_t = [ent(nc.sbuf_tensor(f"out_t{j}", [M, W], F16)) for j in range(2)]
    bnd = ent(nc.sbuf_tensor("bnd", [128, NU * 4 * M], DT))
    pgx = [ent(nc.psum_tensor(f"pgx{j}", [M, W], DT)) for j in range(2)]
    pgy = [ent(nc.psum_tensor(f"pgy{j}", [M, W], DT)) for j in range(2)]

    d_b = ent(nc.semaphore("d_b"))
    d_x = ent(nc.semaphore("d_x"))
    d_sh = ent(nc.semaphore("d_sh"))
    d_out = ent(nc.semaphore("d_out"))
    g_s = ent(nc.semaphore("g_s"))
    g_u = ent(nc.semaphore("g_u"))
    g_h = ent(nc.semaphore("g_h"))
    g_m2 = ent(nc.semaphore("g_m2"))
    v_v = ent(nc.semaphore("v_v"))
    v_w = ent(nc.semaphore("v_w"))
    v_mhv = ent(nc.semaphore("v_mhv"))
    v_d1 = ent(nc.semaphore("v_d1"))
    v_fin = ent(nc.semaphore("v_fin"))
    a_sq = ent(nc.semaphore("a_sq"))
    a_sg = ent(nc.semaphore("a_sg"))
    pe = ent(nc.semaphore("pe"))
    block = ent(nc.Block())

    def IX(u, b):
        return 4 * u + b + 1  # 1-based cumulative count at completion of (u,b)

    @block.sync
    def _(sync):
        sync.dma_start(out=bnd[:], in_=bands[:]).then_inc(d_b, 16)
        for b in range(B):
            sync.dma_start(out=x[b][:], in_=xin[0, b]).then_inc(d_x, 16)
        for u in range(NU):
            # x loads for u+1 BEFORE shifts/outs of u: DVE phaseC(u) comes
            # after phaseA(u+1), which needs Pool phase1(u+1) <- these loads.
            if u + 1 < NU:
                for b in range(B):
                    # x[b] WAR: Pool s2 and DVE w of (u,b) read x[b]
                    sync.wait_ge(g_s, IX(u, b))
                    sync.wait_ge(v_w, IX(u, b))
                    sync.dma_start(out=x[b][:], in_=xin[u + 1, b]).then_inc(d_x, 16)
            # m2 row shifts (after the pair's m2 writes; WAR vs NMS of u-1)
            if u > 0:
                sync.wait_ge(v_fin, 4 * u)
            sync.wait_ge(g_m2, 4 * u + 3)
            sync.dma_start(out=t02p[0:M - 1], in_=t02[1:M]).then_inc(d_sh, 16)
            sync.dma_start(out=t02m[1:M], in_=t02[0:M - 1]).then_inc(d_sh, 16)
            sync.wait_ge(g_m2, 4 * u + 4)
            sync.dma_start(out=t13p[0:M - 1], in_=t13[1:M]).then_inc(d_sh, 16)
            sync.dma_start(out=t13m[1:M], in_=t13[0:M - 1]).then_inc(d_sh, 16)
            for b in range(B):
                sync.wait_ge(v_fin, IX(u, b))
                sync.dma_start(out=outd[u, b],
                               in_=out_t[(4 * u + b) % 2][1:121, :]).then_inc(d_out, 16)
        ndbg = 0
        if dbg is not None:
            sync.wait_ge(v_fin, NU * B)
            sync.dma_start(out=dbg["dbg_t02"][:], in_=t02[:]).then_inc(d_out, 16)
            sync.dma_start(out=dbg["dbg_t13"][:], in_=t13[:]).then_inc(d_out, 16)
            for b in range(B):
                sync.dma_start(out=dbg["dbg_mhv"][b], in_=mhv[b][:]).then_inc(d_out, 16)
                sync.dma_start(out=dbg["dbg_d1m"][b], in_=d1m[b][:]).then_inc(d_out, 16)
            for j in range(2):
                sync.dma_start(out=dbg["dbg_h"][j], in_=Ht[j][:]).then_inc(d_out, 16)
                sync.dma_start(out=dbg["dbg_sq"][j], in_=sqa[j][:]).then_inc(d_out, 16)
            ndbg = 14
        sync.wait_ge(d_out, 16 * (NU * B + ndbg))

    @block.gpsimd
    def _(gpsimd):
        # prologue: zero pads once — H side cols; m2 pad cols; shift edge rows
        for j in range(2):
            gpsimd.memset(Ht[j][:, 0:1], 0.0)
            gpsimd.memset(Ht[j][:, 1025:1026], 0.0)
        for t in (t02, t13):
            gpsimd.memset(t[:, :, 0:1], 0.0)
            gpsimd.memset(t[:, :, 1025:1026], 0.0)
        for t in (t02p, t02m, t13p, t13m):
            gpsimd.memset(t[:, :, 0:1], 0.0)
            gpsimd.memset(t[:, :, 1025:1026], 0.0)
        for u in range(NU):
            # interleave so ut(b) consumes s1[b%2] BEFORE s1(b+2) clobbers it:
            # s1s2(0) s1s2(1) | ut/H(0) s1s2(2) | ut/H(1) s1s2(3) | ut/H(2) ut/H(3)
            def s1s2(b):
                ix = IX(u, b)
                gpsimd.wait_ge(d_x, 16 * ix)
                if 4 * u + b >= 2:
                    gpsimd.wait_ge(v_v, ix - 2)  # s2 slot WAR (DVE v read)
                gpsimd.tensor_tensor(out=s1[b % 2][:], in0=x[b][:, 1:1025],
                                     in1=x[b][:, 3:1027], op=ALU.add)
                gpsimd.tensor_tensor(out=s2[b % 2][:], in0=x[b][:, 0:1024],
                                     in1=x[b][:, 4:1028],
                                     op=ALU.add).then_inc(g_s, 1)

            def uh(b):
                ix = IX(u, b)
                gpsimd.wait_ge(v_v, ix)
                gpsimd.tensor_tensor(out=ut[b % 2][:], in0=vv[b % 2][:],
                                     in1=s1[b % 2][:], op=ALU.add).then_inc(g_u, 1)
                gpsimd.wait_ge(v_w, ix)
                if 4 * u + b >= 2:
                    gpsimd.wait_ge(pe, ix - 2)  # Ht slot WAR (PE read)
                gpsimd.tensor_tensor(out=Ht[b % 2][:, 1:1025], in0=ut[b % 2][:],
                                     in1=ww[b % 2][:], op=ALU.add).then_inc(g_h, 1)

            s1s2(0)
            s1s2(1)
            uh(0)
            s1s2(2)
            uh(1)
            s1s2(3)
            uh(2)
            uh(3)
            for b in range(B):
                ix = IX(u, b)
                gpsimd.wait_ge(a_sq, ix)
                if u > 0 and b == 0:
                    gpsimd.wait_ge(v_fin, 4 * u)   # m2 tiles WAR (NMS of u-1)
                    gpsimd.wait_ge(d_sh, 64 * u)   # and shift DMAs of u-1
                tile = t02 if b % 2 == 0 else t13
                q = b // 2
                gpsimd.tensor_tensor(out=tile[:, q, 1:1025], in0=tsqx[b % 2][:],
                                     in1=tsqy[b % 2][:],
                                     op=ALU.add).then_inc(g_m2, 1)

    @block.vector
    def _(vector):
        def phaseA(u):
            for b in range(B):
                ix = IX(u, b)
                vector.wait_ge(g_s, ix)
                if 4 * u + b >= 2:
                    vector.wait_ge(g_u, ix - 2)  # vv slot WAR (Pool read)
                vector.tensor_scalar(out=vv[b % 2][:], in0=s2[b % 2][:],
                                     scalar1=R0, scalar2=None,
                                     op0=ALU.mult).then_inc(v_v, 1)
                vector.wait_ge(d_x, 16 * ix)
                if 4 * u + b >= 2:
                    vector.wait_ge(g_h, ix - 2)  # ww slot WAR (Pool read)
                vector.tensor_scalar(out=ww[b % 2][:], in0=x[b][:, 2:1026],
                                     scalar1=R2, scalar2=None,
                                     op0=ALU.mult).then_inc(v_w, 1)

        def phaseB(u):
            for b in range(B):
                ix = IX(u, b)
                vector.wait_ge(a_sq, ix)
                vector.tensor_tensor(out=mhv[b][:], in0=sqa[b % 2][:, 0:2048],
                                     in1=sqa[b % 2][:, 1024:3072],
                                     op=ALU.is_ge).then_inc(v_mhv, 1)
                vector.wait_ge(a_sg, ix)
                vector.tensor_tensor(out=d1m[b][:], in0=sgx[b % 2][:],
                                     in1=sgy[b % 2][:],
                                     op=ALU.is_equal).then_inc(v_d1, 1)

        def phaseC(u):
            vector.wait_ge(d_sh, 64 * (u + 1))
            for b in range(B):
                ix = IX(u, b)
                if b == 0:
                    pv, mv_ = (t02[:, :, 2:1026], t13[:, :, 2:1026]), \
                              (t02[:, :, 0:1024], t13[:, :, 0:1024])
                elif b == 1:
                    pv, mv_ = (t02p[:, :, 2:1026], t13p[:, :, 2:1026]), \
                              (t02m[:, :, 0:1024], t13m[:, :, 0:1024])
                elif b == 2:
                    pv, mv_ = (t02p[:, :, 1:1025], t13p[:, :, 1:1025]), \
                              (t02m[:, :, 1:1025], t13m[:, :, 1:1025])
                else:
                    pv, mv_ = (t02p[:, :, 0:1024], t13p[:, :, 0:1024]), \
                              (t02m[:, :, 2:1026], t13m[:, :, 2:1026])
                c02 = t02[:, :, 1:1025]
                c13 = t13[:, :, 1:1025]
                vector.tensor_tensor(out=cn13[:], in0=c13, in1=mv_[1], op=ALU.is_gt)
                vector.tensor_tensor(out=cn02[:], in0=c02, in1=mv_[0], op=ALU.is_gt)
                vector.tensor_tensor(out=cp13[:], in0=c13, in1=pv[1], op=ALU.is_gt)
                vector.tensor_tensor(out=cp02[:], in0=c02, in1=pv[0], op=ALU.is_gt)
                # candidates: default=(c1>m1)&(c3>m3); d1=(c0>m0)&(c2>m2);
                #             mv=(c1>p1)&(c3>p3);      mh=(c0>p0)&(c2>p2)
                vector.tensor_tensor(out=s12[:], in0=cn13[:, 0, :],
                                     in1=cn13[:, 1, :], op=ALU.mult)
                vector.tensor_tensor(out=aD[:], in0=cn02[:, 0, :],
                                     in1=cn02[:, 1, :], op=ALU.mult)
                vector.tensor_tensor(out=aV[:], in0=cp13[:, 0, :],
                                     in1=cp13[:, 1, :], op=ALU.mult)
                vector.tensor_tensor(out=aH[:], in0=cp02[:, 0, :],
                                     in1=cp02[:, 1, :], op=ALU.mult)
                vector.copy_predicated(out=s12[:], mask=d1m[b][:], data=aD[:])
                vector.copy_predicated(out=s12[:], mask=mhv[b][:, 1024:2048],
                                       data=aV[:])
                vector.copy_predicated(out=s12[:], mask=mhv[b][:, 0:1024],
                                       data=aH[:])
                tile = t02 if b % 2 == 0 else t13
                q = b // 2
                vector.tensor_scalar(out=thr_t[:], in0=tile[:, q, 1:1025],
                                     scalar1=THR, scalar2=None, op0=ALU.is_ge)
                if 4 * u + b >= 2:
                    vector.wait_ge(d_out, 16 * (ix - 2))
                vector.tensor_tensor(out=out_t[(4 * u + b) % 2][:], in0=thr_t[:],
                                     in1=s12[:], op=ALU.mult).then_inc(v_fin, 1)

        # schedule: A(0) B(0) A(1) C(0) B(1) A(2) C(1) B(2) A(3) C(2) B(3) C(3)
        phaseA(0)
        phaseB(0)
        for u in range(1, NU):
            phaseA(u)
            phaseC(u - 1)
            phaseB(u)
        phaseC(NU - 1)

    @block.scalar
    def _(scalar):
        for u in range(NU):
            for b in range(B):
                ix = IX(u, b)
                scalar.wait_ge(pe, ix)
                if 4 * u + b >= 2:
                    scalar.wait_ge(v_mhv, ix - 2)
                    scalar.wait_ge(g_m2, ix - 2)
                    scalar.wait_ge(v_d1, ix - 2)
                j = b % 2
                p, q = pgx[j], pgy[j]
                nc.scalar.activation(out=sqa[j][:, 0:1024], in_=p[:],
                                     func=AF.Square, scale=_t1)
                nc.scalar.activation(out=sqa[j][:, 1024:2048], in_=q[:],
                                     func=AF.Square)
                nc.scalar.activation(out=sqa[j][:, 2048:3072], in_=p[:],
                                     func=AF.Square, scale=_t3)
                nc.scalar.activation(out=tsqx[j][:], in_=p[:],
                                     func=AF.Square)
                nc.scalar.activation(out=tsqy[j][:], in_=q[:],
                                     func=AF.Square).then_inc(a_sq, 1)
                nc.scalar.activation(out=sgx[j][:], in_=p[:], func=AF.Sign)
                nc.scalar.activation(out=sgy[j][:], in_=q[:],
                                     func=AF.Sign).then_inc(a_sg, 1)

    @block.tensor
    def _(tensor):
        tensor.wait_ge(d_b, 16)
        for u in range(NU):
            off = u * 4 * M
            bxp = bnd[:, off + 0 * M:off + 1 * M]
            bxn = bnd[:, off + 1 * M:off + 2 * M]
            by1 = bnd[:, off + 2 * M:off + 3 * M]
            by2 = bnd[:, off + 3 * M:off + 4 * M]
            for b in range(B):
                ix = IX(u, b)
                tensor.wait_ge(g_h, ix)
                if 4 * u + b >= 2:
                    tensor.wait_ge(a_sg, ix - 2)  # PSUM pair WAR (Act reads)
                j = b % 2
                p, q, H_ = pgx[j], pgy[j], Ht[j]
                mm = nc.tensor.matmul
                mm(p[:, 0:512], bxp, H_[:, 0:512], start=True, stop=False)
                mm(p[:, 512:1024], bxp, H_[:, 512:1024], start=True, stop=False)
                mm(p[:, 0:512], bxn, H_[:, 2:514], start=False, stop=True)
                mm(p[:, 512:1024], bxn, H_[:, 514:1026], start=False, stop=True)
                mm(q[:, 0:512], by1, H_[:, 0:512], start=True, stop=False)
                mm(q[:, 512:1024], by1, H_[:, 512:1024], start=True, stop=False)
                mm(q[:, 0:512], by1, H_[:, 2:514], start=False, stop=False)
                mm(q[:, 512:1024], by1, H_[:, 514:1026], start=False, stop=False)
                mm(q[:, 0:512], by2, H_[:, 1:513], start=False, stop=True)
                mm(q[:, 512:1024], by2, H_[:, 513:1025], start=False,
                   stop=True).then_inc(pe, 1)

    es.close()
    return nc


_NC_CACHE = {}


def kernel(img, gauss_h=None, gauss_v=None, sobel_h=None, sobel_v=None,
           dir_w=None, **_):
    img = np.asarray(img, dtype=np.float32)
    assert img.shape == (B, C, H_IMG, W)

    # host pad with 0 and fold the (+1) affine in
    pad = np.zeros((B, C, H_IMG + 8, W + 4), np.float32)
    pad[:, :, 4:4 + H_IMG, 2:2 + W] = img + 1.0

    CX, CY = _make_bands()
    band_cache = {}
    for c, k in UNITS:
        if k not in band_cache:
            bx = _band_lhsT(CX, k)
            by = _band_lhsT(CY, k)
            band_cache[k] = (bx, -bx, by, 2.0 * by)

    in_maps = []
    for i in range(8):
        xin = np.empty((NU, B, XR, FW), np.float32)
        bands = np.zeros((128, NU * 4 * M), np.float32)
        for u, (c, k) in enumerate(CORE_UNITS[i][:NU]):
            xbase, _o = _unit_rows(k)
            r = xbase + 4  # padded row index
            for b in range(B):
                xin[u, b] = pad[b, c, r:r + XR, :]
            for t, bb in enumerate(band_cache[k]):
                bands[:, (u * 4 + t) * M:(u * 4 + t + 1) * M] = bb
        in_maps.append({"xin": xin, "bands": bands})

    key = "nc"
    if key not in _NC_CACHE:
        _NC_CACHE[key] = build_nc()
    nc = _NC_CACHE[key]
    r = run_bass_kernel_spmd(nc, in_maps, list(range(8)))
    globals()["LAST_RESULT"] = r
    res = r.results

    out = np.zeros((B, C, H_IMG, W), np.float32)
    for i in range(8):
        for u, (c, k) in enumerate(CORE_UNITS[i][:NU]):
            if i >= 3 and u == 3:
                continue  # dummy repeat
            _xb, out0 = _unit_rows(k)
            rows = 120 if k < 8 else 64
            out[:, c, out0:out0 + rows, :] = \
                res[i]["out"][u, :, :rows, :].astype(np.float32)
    mn, mx = out.min(), out.max()
    return ((out - mn) / (mx - mn)).astype(np.float32)


# revision 12
# speedup vs baseline: 1.8841x; 1.0771x over previous
"""Canny edge detector (nn_CannyNet) on 8 Trainium2 NeuronCores.

Self-contained: hardcodes shapes [4,3,1024,1024] and the filter constants.

Decomposition: 27 units = (3 channels) x (9 row-blocks: 8x120 + 1x64 rows).
Uniform SPMD program: every core processes 4 units (5 cores repeat their
first unit; host ignores the duplicate). Per unit, the 4 batch planes of one
channel are processed together because the reference's flat NMS gather
couples batches.

Engine split (per plane, [<=128, 1024] tiles):
  Pool : s1/s2 col shift-adds, u=v+s1, H=u+w, m2=sqx+sqy (plain TT adds only)
  DVE  : v=s2*R0, w=x*R2 (fast TS), masks + fp16 NMS compare cascade
  PE   : gx/gy directly via accumulating banded matmuls with col-shifted
         rhs views of the zero-padded H tile (vertical conv x horizontal taps)
  Act  : squares (with tan-scale folding) + signs straight from PSUM, fp16 out
All NMS compares run in fp16 (2x DVE rate), planes packed in pairs (1,3)/(0,2)
so one compare instruction covers both planes of an AND-candidate.
Row-shifted m2 views (m2p/m2m) via SBUF->SBUF DMA (engines cannot shift
partitions). Final out is fp16 0/1, converted on host.
"""
import math
import os
import numpy as np

import concourse.bass as bass
import concourse.mybir as mybir
from concourse.bass_utils import run_bass_kernel_spmd

ALU = mybir.AluOpType
AF = mybir.ActivationFunctionType
DT = mybir.dt.float32
F16 = mybir.dt.float16
U16 = mybir.dt.uint16

B, C, H_IMG, W = 4, 3, 1024, 1024
NU = 1 if os.environ.get('KDBG') else 4  # units per core (uniform)
M = 122           # m2/out row span per unit (out 120 + 2)
XR = 128          # x-tile rows
FW = 1028         # x-tile width (cols -2..1025)
HW2 = 1026        # H tile width (cols -1..1024, zero side cols)
MW = 1026         # m2 width per plane (cols -1..1024)

_g = np.exp(-0.5 * np.arange(-2, 3, dtype=np.float64) ** 2)
G1 = _g[1]
R0 = float(np.float32(_g[0] / _g[1]))   # g0/g1
R2 = float(np.float32(1.0 / _g[1]))     # 1/g1
THR = float(np.float32((400.0 / (127.5 * G1)) ** 2))
_t1 = math.tan(22.5 * 3.14159 / 180.0)
_t3 = math.tan(67.5 * 3.14159 / 180.0)

# units and core assignment
UNITS = [(c, k) for k in range(9) for c in range(3)]  # 27
CORE_UNITS = []
for i in range(8):
    us = [UNITS[i], UNITS[i + 8], UNITS[i + 16]]
    us.append(UNITS[24 + i] if i < 3 else UNITS[i])  # dummy repeat for cores 3..7
    CORE_UNITS.append(us)


def _unit_rows(k):
    """(xbase, out0): x-tile img rows xbase..xbase+127; out rows out0..out0+119
    (k=8: only first 64 valid)."""
    if k < 8:
        return 120 * k - 4, 120 * k
    return 900, 960


def _make_bands():
    """CX = S121 @ G, CY = S101 @ G over image rows with zero-pad truncation."""
    n = H_IMG
    G = np.zeros((n, n), np.float64)
    for kk in range(-2, 3):
        v = _g[kk + 2]
        for o in range(max(0, -kk), min(n, n - kk)):
            G[o, o + kk] = v
    S121 = np.zeros((n, n), np.float64)
    S101 = np.zeros((n, n), np.float64)
    for o in range(n):
        for kk, w1, w2 in ((-1, 1.0, 1.0), (0, 2.0, 0.0), (1, 1.0, -1.0)):
            i = o + kk
            if 0 <= i < n:
                S121[o, i] = w1
                if kk != 0:
                    S101[o, i] = w2
    CX = (S121 @ G).astype(np.float32)
    CY = (S101 @ G).astype(np.float32)
    return CX, CY


def _band_lhsT(Cm, k):
    """lhsT [XR, M]: lhsT[kr, m] = Cm[out0-1+m, xbase+kr] (0 out of range)."""
    xbase, out0 = _unit_rows(k)
    out = np.zeros((XR, M), np.float32)
    for m in range(M):
        orow = out0 - 1 + m
        if not (0 <= orow < H_IMG):
            continue
        for d in range(-3, 4):
            irow = orow + d
            kr = irow - xbase
            if 0 <= irow < H_IMG and 0 <= kr < XR:
                out[kr, m] = Cm[orow, irow]
    return out


def build_nc():
    nc = bass.Bass()
    xin = nc.declare_dram_parameter("xin", [NU, B, XR, FW], DT, isOutput=False)
    bands = nc.declare_dram_parameter("bands", [128, NU * 4 * M], DT, isOutput=False)
    outd = nc.declare_dram_parameter("out", [NU, B, 120, W], F16, isOutput=True)
    dbg = None
    if os.environ.get('KDBG'):
        dbg = {
            "dbg_t02": nc.declare_dram_parameter("dbg_t02", [M, 2, MW], DT, isOutput=True),
            "dbg_t13": nc.declare_dram_parameter("dbg_t13", [M, 2, MW], DT, isOutput=True),
            "dbg_mhv": nc.declare_dram_parameter("dbg_mhv", [B, M, 2 * W], U16, isOutput=True),
            "dbg_d1m": nc.declare_dram_parameter("dbg_d1m", [B, M, W], U16, isOutput=True),
            "dbg_h": nc.declare_dram_parameter("dbg_h", [2, XR, HW2], DT, isOutput=True),
            "dbg_sq": nc.declare_dram_parameter("dbg_sq", [2, M, 3 * W], F16, isOutput=True),
        }

    from contextlib import ExitStack
    es = ExitStack()
    ent = es.enter_context

    x = [ent(nc.sbuf_tensor(f"x{b}", [XR, FW], DT)) for b in range(B)]
    s1 = [ent(nc.sbuf_tensor(f"s1_{j}", [XR, W], DT)) for j in range(2)]
    s2 = [ent(nc.sbuf_tensor(f"s2_{j}", [XR, W], DT)) for j in range(2)]
    vv = [ent(nc.sbuf_tensor(f"vv{j}", [XR, W], DT)) for j in range(2)]
    ww = [ent(nc.sbuf_tensor(f"ww{j}", [XR, W], DT)) for j in range(2)]
    Ht = [ent(nc.sbuf_tensor(f"Ht{j}", [XR, HW2], DT)) for j in range(2)]
    ut = s2  # ut aliases s2: s2[j] is dead once DVE v(b) = s2*R0 has read it
    # fp16 working set
    sqa = [ent(nc.sbuf_tensor(f"sqa{j}", [M, 3 * W], F16)) for j in range(2)]
    tsqx = [ent(nc.sbuf_tensor(f"tsqx{j}", [M, W], DT)) for j in range(2)]
    tsqy = [ent(nc.sbuf_tensor(f"tsqy{j}", [M, W], DT)) for j in range(2)]
    sgx = [ent(nc.sbuf_tensor(f"sgx{j}", [M, W], F16)) for j in range(2)]
    sgy = [ent(nc.sbuf_tensor(f"sgy{j}", [M, W], F16)) for j in range(2)]
    mhv = [ent(nc.sbuf_tensor(f"mhv{b}", [M, 2 * W], U16)) for b in range(B)]
    d1m = [ent(nc.sbuf_tensor(f"d1m{b}", [M, W], U16)) for b in range(B)]
    # packed m2 tiles: [M, 2, MW] — slice 0/1 = planes (0,2) in t02, (1,3) in t13
    t02 = ent(nc.sbuf_tensor("t02", [M, 2, MW], DT))
    t13 = ent(nc.sbuf_tensor("t13", [M, 2, MW], DT))
    t02p = ent(nc.sbuf_tensor("t02p", [M, 2, MW], DT))
    t02m = ent(nc.sbuf_tensor("t02m", [M, 2, MW], DT))
    t13p = ent(nc.sbuf_tensor("t13p", [M, 2, MW], DT))
    t13m = ent(nc.sbuf_tensor("t13m", [M, 2, MW], DT))
    cn13 = ent(nc.sbuf_tensor("cn13", [M, 2, W], F16))
    cn02 = ent(nc.sbuf_tensor("cn02", [M, 2, W], F16))
    cp13 = ent(nc.sbuf_tensor("cp13", [M, 2, W], F16))
    cp02 = ent(nc.sbuf_tensor("cp02", [M, 2, W], F16))
    s12 = ent(nc.sbuf_tensor("s12", [M, W], F16))
    aD = ent(nc.sbuf_tensor("aD", [M, W], F16))
    aV = ent(nc.sbuf_tensor("aV", [M, W], F16))
    aH = ent(nc.sbuf_tensor("aH", [M, W], F16))
    thr_t = ent(nc.sbuf_tensor("thr_t", [M, W], F16))
    out_t = [ent(nc.sbuf_tensor(f"out_t{j}", [M, W], F16)) for j in range(2)]
    bnd = ent(nc.sbuf_tensor("bnd", [128, NU * 4 * M], DT))
    pgx = [ent(nc.psum_tensor(f"pgx{j}", [M, W], DT)) for j in range(2)]
    pgy = [ent(nc.psum_tensor(f"pgy{j}", [M, W], DT)) for j in range(2)]

    d_b = ent(nc.semaphore("d_b"))
    d_x = ent(nc.semaphore("d_x"))
    d_sh = ent(nc.semaphore("d_sh"))
    d_out = ent(nc.semaphore("d_out"))
    g_s = ent(nc.semaphore("g_s"))
    g_u = ent(nc.semaphore("g_u"))
    g_h = ent(nc.semaphore("g_h"))
    g_m2 = ent(nc.semaphore("g_m2"))
    v_v = ent(nc.semaphore("v_v"))
    v_w = ent(nc.semaphore("v_w"))
    v_mhv = ent(nc.semaphore("v_mhv"))
    v_d1 = ent(nc.semaphore("v_d1"))
    v_fin = ent(nc.semaphore("v_fin"))
    a_sq = ent(nc.semaphore("a_sq"))
    a_sg = ent(nc.semaphore("a_sg"))
    pe = ent(nc.semaphore("pe"))
    block = ent(nc.Block())

    def IX(u, b):
        return 4 * u + b + 1  # 1-based cumulative count at completion of (u,b)

    @block.sync
    def _(sync):
        sync.dma_start(out=bnd[:], in_=bands[:]).then_inc(d_b, 16)
        for b in range(B):
            sync.dma_start(out=x[b][:], in_=xin[0, b]).then_inc(d_x, 16)
        for u in range(NU):
            # x loads for u+1 BEFORE shifts/outs of u: DVE phaseC(u) comes
            # after phaseA(u+1), which needs Pool phase1(u+1) <- these loads.
            if u + 1 < NU:
                for b in range(B):
                    # x[b] WAR: Pool s2 and DVE w of (u,b) read x[b]
                    sync.wait_ge(g_s, IX(u, b))
                    sync.wait_ge(v_w, IX(u, b))
                    sync.dma_start(out=x[b][:], in_=xin[u + 1, b]).then_inc(d_x, 16)
            # per-plane m2 row shifts, launched as soon as each slice lands
            if u > 0:
                sync.wait_ge(v_fin, 4 * u)  # WAR: NMS of u-1 read shift tiles
            for b in range(B):
                sync.wait_ge(g_m2, IX(u, b))
                src = t02 if b % 2 == 0 else t13
                tp = t02p if b % 2 == 0 else t13p
                tm = t02m if b % 2 == 0 else t13m
                q = b // 2
                sync.dma_start(out=tp[0:M - 1, q], in_=src[1:M, q]).then_inc(d_sh, 16)
                sync.dma_start(out=tm[1:M, q], in_=src[0:M - 1, q]).then_inc(d_sh, 16)
            for b in range(B):
                sync.wait_ge(v_fin, IX(u, b))
                sync.dma_start(out=outd[u, b],
                               in_=out_t[(4 * u + b) % 2][1:121, :]).then_inc(d_out, 16)
        ndbg = 0
        if dbg is not None:
            sync.wait_ge(v_fin, NU * B)
            sync.dma_start(out=dbg["dbg_t02"][:], in_=t02[:]).then_inc(d_out, 16)
            sync.dma_start(out=dbg["dbg_t13"][:], in_=t13[:]).then_inc(d_out, 16)
            for b in range(B):
                sync.dma_start(out=dbg["dbg_mhv"][b], in_=mhv[b][:]).then_inc(d_out, 16)
                sync.dma_start(out=dbg["dbg_d1m"][b], in_=d1m[b][:]).then_inc(d_out, 16)
            for j in range(2):
                sync.dma_start(out=dbg["dbg_h"][j], in_=Ht[j][:]).then_inc(d_out, 16)
                sync.dma_start(out=dbg["dbg_sq"][j], in_=sqa[j][:]).then_inc(d_out, 16)
            ndbg = 14
        sync.wait_ge(d_out, 16 * (NU * B + ndbg))

    @block.gpsimd
    def _(gpsimd):
        # prologue: zero pads once — H side cols; m2 pad cols; shift edge rows
        for j in range(2):
            gpsimd.memset(Ht[j][:, 0:1], 0.0)
            gpsimd.memset(Ht[j][:, 1025:1026], 0.0)
        for t in (t02, t13):
            gpsimd.memset(t[:, :, 0:1], 0.0)
            gpsimd.memset(t[:, :, 1025:1026], 0.0)
        for t in (t02p, t02m, t13p, t13m):
            gpsimd.memset(t[:, :, 0:1], 0.0)
            gpsimd.memset(t[:, :, 1025:1026], 0.0)
        for u in range(NU):
            # interleave so ut(b) consumes s1[b%2] BEFORE s1(b+2) clobbers it:
            # s1s2(0) s1s2(1) | ut/H(0) s1s2(2) | ut/H(1) s1s2(3) | ut/H(2) ut/H(3)
            def s1s2(b):
                ix = IX(u, b)
                gpsimd.wait_ge(d_x, 16 * ix)
                if 4 * u + b >= 2:
                    gpsimd.wait_ge(v_v, ix - 2)  # s2 slot WAR (DVE v read)
                gpsimd.tensor_tensor(out=s1[b % 2][:], in0=x[b][:, 1:1025],
                                     in1=x[b][:, 3:1027], op=ALU.add)
                gpsimd.tensor_tensor(out=s2[b % 2][:], in0=x[b][:, 0:1024],
                                     in1=x[b][:, 4:1028],
                                     op=ALU.add).then_inc(g_s, 1)

            def uh(b):
                ix = IX(u, b)
                gpsimd.wait_ge(v_v, ix)
                gpsimd.tensor_tensor(out=ut[b % 2][:], in0=vv[b % 2][:],
                                     in1=s1[b % 2][:], op=ALU.add).then_inc(g_u, 1)
                gpsimd.wait_ge(v_w, ix)
                if 4 * u + b >= 2:
                    gpsimd.wait_ge(pe, ix - 2)  # Ht slot WAR (PE read)
                gpsimd.tensor_tensor(out=Ht[b % 2][:, 1:1025], in0=ut[b % 2][:],
                                     in1=ww[b % 2][:], op=ALU.add).then_inc(g_h, 1)

            s1s2(0)
            s1s2(1)
            uh(0)
            s1s2(2)
            uh(1)
            s1s2(3)
            uh(2)
            uh(3)
            for b in range(B):
                ix = IX(u, b)
                gpsimd.wait_ge(a_sq, ix)
                if u > 0 and b == 0:
                    gpsimd.wait_ge(v_fin, 4 * u)   # m2 tiles WAR (NMS of u-1)
                    gpsimd.wait_ge(d_sh, 64 * u)   # and shift DMAs of u-1
                tile = t02 if b % 2 == 0 else t13
                q = b // 2
                gpsimd.tensor_tensor(out=tile[:, q, 1:1025], in0=tsqx[b % 2][:],
                                     in1=tsqy[b % 2][:],
                                     op=ALU.add).then_inc(g_m2, 1)

    @block.vector
    def _(vector):
        def phaseB(u):
            for b in range(B):
                ix = IX(u, b)
                vector.wait_ge(a_sq, ix)
                vector.tensor_tensor(out=mhv[b][:], in0=sqa[b % 2][:, 0:2048],
                                     in1=sqa[b % 2][:, 1024:3072],
                                     op=ALU.is_ge).then_inc(v_mhv, 1)
                vector.wait_ge(a_sg, ix)
                vector.tensor_tensor(out=d1m[b][:], in0=sgx[b % 2][:],
                                     in1=sgy[b % 2][:],
                                     op=ALU.is_equal).then_inc(v_d1, 1)

        def phaseC(u):
            # b=0 (E/W) needs no row-shift tiles: only all m2 slices written
            vector.wait_ge(g_m2, 4 * (u + 1))
            for b in range(B):
                ix = IX(u, b)
                if b == 1:
                    vector.wait_ge(d_sh, 32 * B * (u + 1))
                if b == 0:
                    pv, mv_ = (t02[:, :, 2:1026], t13[:, :, 2:1026]), \
                              (t02[:, :, 0:1024], t13[:, :, 0:1024])
                elif b == 1:
                    pv, mv_ = (t02p[:, :, 2:1026], t13p[:, :, 2:1026]), \
                              (t02m[:, :, 0:1024], t13m[:, :, 0:1024])
                elif b == 2:
                    pv, mv_ = (t02p[:, :, 1:1025], t13p[:, :, 1:1025]), \
                              (t02m[:, :, 1:1025], t13m[:, :, 1:1025])
                else:
                    pv, mv_ = (t02p[:, :, 0:1024], t13p[:, :, 0:1024]), \
                              (t02m[:, :, 2:1026], t13m[:, :, 2:1026])
                c02 = t02[:, :, 1:1025]
                c13 = t13[:, :, 1:1025]
                vector.tensor_tensor(out=cn13[:], in0=c13, in1=mv_[1], op=ALU.is_gt)
                vector.tensor_tensor(out=cn02[:], in0=c02, in1=mv_[0], op=ALU.is_gt)
                vector.tensor_tensor(out=cp13[:], in0=c13, in1=pv[1], op=ALU.is_gt)
                vector.tensor_tensor(out=cp02[:], in0=c02, in1=pv[0], op=ALU.is_gt)
                # candidates: default=(c1>m1)&(c3>m3); d1=(c0>m0)&(c2>m2);
                #             mv=(c1>p1)&(c3>p3);      mh=(c0>p0)&(c2>p2)
                vector.tensor_tensor(out=s12[:], in0=cn13[:, 0, :],
                                     in1=cn13[:, 1, :], op=ALU.mult)
                vector.tensor_tensor(out=aD[:], in0=cn02[:, 0, :],
                                     in1=cn02[:, 1, :], op=ALU.mult)
                vector.tensor_tensor(out=aV[:], in0=cp13[:, 0, :],
                                     in1=cp13[:, 1, :], op=ALU.mult)
                vector.tensor_tensor(out=aH[:], in0=cp02[:, 0, :],
                                     in1=cp02[:, 1, :], op=ALU.mult)
                vector.copy_predicated(out=s12[:], mask=d1m[b][:], data=aD[:])
                vector.copy_predicated(out=s12[:], mask=mhv[b][:, 1024:2048],
                                       data=aV[:])
                vector.copy_predicated(out=s12[:], mask=mhv[b][:, 0:1024],
                                       data=aH[:])
                tile = t02 if b % 2 == 0 else t13
                q = b // 2
                vector.tensor_scalar(out=thr_t[:], in0=tile[:, q, 1:1025],
                                     scalar1=THR, scalar2=None, op0=ALU.is_ge)
                if 4 * u + b >= 2:
                    vector.wait_ge(d_out, 16 * (ix - 2))
                vector.tensor_tensor(out=out_t[(4 * u + b) % 2][:], in0=thr_t[:],
                                     in1=s12[:], op=ALU.mult).then_inc(v_fin, 1)

        for u in range(NU):
            phaseB(u)
            phaseC(u)

    @block.scalar
    def _(scalar):
        for u in range(NU):
            for b in range(B):
                ix = IX(u, b)
                j = b % 2
                scalar.wait_ge(d_x, 16 * ix)
                if 4 * u + b >= 2:
                    scalar.wait_ge(g_h, ix - 2)  # ww slot WAR (Pool H read)
                nc.scalar.activation(out=ww[j][:], in_=x[b][:, 2:1026],
                                     func=AF.Copy, scale=R2).then_inc(v_w, 1)
                scalar.wait_ge(g_s, ix)
                if 4 * u + b >= 2:
                    scalar.wait_ge(g_u, ix - 2)  # vv slot WAR (Pool ut read)
                nc.scalar.activation(out=vv[j][:], in_=s2[j][:],
                                     func=AF.Copy, scale=R0).then_inc(v_v, 1)
            for b in range(B):
                ix = IX(u, b)
                scalar.wait_ge(pe, ix)
                if 4 * u + b >= 2:
                    scalar.wait_ge(v_mhv, ix - 2)
                    scalar.wait_ge(g_m2, ix - 2)
                    scalar.wait_ge(v_d1, ix - 2)
                j = b % 2
                p, q = pgx[j], pgy[j]
                nc.scalar.activation(out=sqa[j][:, 0:1024], in_=p[:],
                                     func=AF.Square, scale=_t1)
                nc.scalar.activation(out=sqa[j][:, 1024:2048], in_=q[:],
                                     func=AF.Square)
                nc.scalar.activation(out=sqa[j][:, 2048:3072], in_=p[:],
                                     func=AF.Square, scale=_t3)
                nc.scalar.activation(out=tsqx[j][:], in_=p[:],
                                     func=AF.Square)
                nc.scalar.activation(out=tsqy[j][:], in_=q[:],
                                     func=AF.Square).then_inc(a_sq, 1)
                nc.scalar.activation(out=sgx[j][:], in_=p[:], func=AF.Sign)
                nc.scalar.activation(out=sgy[j][:], in_=q[:],
                                     func=AF.Sign).then_inc(a_sg, 1)

    @block.tensor
    def _(tensor):
        tensor.wait_ge(d_b, 16)
        for u in range(NU):
            off = u * 4 * M
            bxp = bnd[:, off + 0 * M:off + 1 * M]
            bxn = bnd[:, off + 1 * M:off + 2 * M]
            by1 = bnd[:, off + 2 * M:off + 3 * M]
            by2 = bnd[:, off + 3 * M:off + 4 * M]
            for b in range(B):
                ix = IX(u, b)
                tensor.wait_ge(g_h, ix)
                if 4 * u + b >= 2:
                    tensor.wait_ge(a_sg, ix - 2)  # PSUM pair WAR (Act reads)
                j = b % 2
                p, q, H_ = pgx[j], pgy[j], Ht[j]
                mm = nc.tensor.matmul
                mm(p[:, 0:512], bxp, H_[:, 0:512], start=True, stop=False)
                mm(p[:, 512:1024], bxp, H_[:, 512:1024], start=True, stop=False)
                mm(p[:, 0:512], bxn, H_[:, 2:514], start=False, stop=True)
                mm(p[:, 512:1024], bxn, H_[:, 514:1026], start=False, stop=True)
                mm(q[:, 0:512], by1, H_[:, 0:512], start=True, stop=False)
                mm(q[:, 512:1024], by1, H_[:, 512:1024], start=True, stop=False)
                mm(q[:, 0:512], by1, H_[:, 2:514], start=False, stop=False)
                mm(q[:, 512:1024], by1, H_[:, 514:1026], start=False, stop=False)
                mm(q[:, 0:512], by2, H_[:, 1:513], start=False, stop=True)
                mm(q[:, 512:1024], by2, H_[:, 513:1025], start=False,
                   stop=True).then_inc(pe, 1)

    es.close()
    return nc


_NC_CACHE = {}


def kernel(img, gauss_h=None, gauss_v=None, sobel_h=None, sobel_v=None,
           dir_w=None, **_):
    img = np.asarray(img, dtype=np.float32)
    assert img.shape == (B, C, H_IMG, W)

    # host pad with 0 and fold the (+1) affine in
    pad = np.zeros((B, C, H_IMG + 8, W + 4), np.float32)
    pad[:, :, 4:4 + H_IMG, 2:2 + W] = img + 1.0

    CX, CY = _make_bands()
    band_cache = {}
    for c, k in UNITS:
        if k not in band_cache:
            bx = _band_lhsT(CX, k)
            by = _band_lhsT(CY, k)
            band_cache[k] = (bx, -bx, by, 2.0 * by)

    in_maps = []
    for i in range(8):
        xin = np.empty((NU, B, XR, FW), np.float32)
        bands = np.zeros((128, NU * 4 * M), np.float32)
        for u, (c, k) in enumerate(CORE_UNITS[i][:NU]):
            xbase, _o = _unit_rows(k)
            r = xbase + 4  # padded row index
            for b in range(B):
                xin[u, b] = pad[b, c, r:r + XR, :]
            for t, bb in enumerate(band_cache[k]):
                bands[:, (u * 4 + t) * M:(u * 4 + t + 1) * M] = bb
        in_maps.append({"xin": xin, "bands": bands})

    key = "nc"
    if key not in _NC_CACHE:
        _NC_CACHE[key] = build_nc()
    nc = _NC_CACHE[key]
    r = run_bass_kernel_spmd(nc, in_maps, list(range(8)))
    globals()["LAST_RESULT"] = r
    res = r.results

    out = np.zeros((B, C, H_IMG, W), np.float32)
    for i in range(8):
        for u, (c, k) in enumerate(CORE_UNITS[i][:NU]):
            if i >= 3 and u == 3:
                continue  # dummy repeat
            _xb, out0 = _unit_rows(k)
            rows = 120 if k < 8 else 64
            out[:, c, out0:out0 + rows, :] = \
                res[i]["out"][u, :, :rows, :].astype(np.float32)
    mn, mx = out.min(), out.max()
    return ((out - mn) / (mx - mn)).astype(np.float32)
